# revision 1
# baseline (speedup 1.0000x reference)
"""Trainium2 Bass kernel for BatchAllTripletWithClustersLossSemiHard.

Strategy (data-parallel over anchors, 8 cores):
  For anchor i only same-label j matter (~B/NCLASS of them).  Build a compact
  (i,j) pair list per core (anchors greedily balanced across cores); each
  pair is one SBUF partition row over k=0..B-1:
      z[p,k] = w_j*(1 + d_ij - d_ik) + mask,   W = relu(z)
  All label logic (semi-hard rank masks, k==i exclusion, padding) is
  precomputed on host into an additive bf16 mask tensor madd (-1e8 masked,
  0 live, +1 sentinel at k=j).  d_ij - d_ik = V_ik - V_ij with
  V[i,k] = 2*x_i.x_k - |x_k|^2 (|x_i|^2 cancels); V is computed on device
  with PE matmuls.  Pair rows are broadcast from V by a per-pair-weight-
  scaled one-hot selector matmul on the PE (so the w_j multiply is free).
  The per-pair scalar w*V_ij is recovered from the sentinel column with one
  fused DVE dot; the sentinel contributes exactly (w+1) to the sum and 1 to
  the count, which the host subtracts.  One ScalarE activation per row-chunk
  produces sum(relu(z)) per partition; counts alternate between a DVE is_gt
  and a ScalarE Sign pass to balance engines.  The 8 per-core partials
  combine on host: loss = S / (C + eps).
"""

import numpy as np

import concourse.bass as bass
import concourse.tile as tile
from concourse import bacc, mybir
from concourse.bass_utils import run_bass_kernel_spmd

EPS = 1e-8
NEG = -1.0e8
B, D, NCORES = 384, 512, 8
PERCORE = B // NCORES  # 48
P = 128
DT = mybir.dt.float32
BF = mybir.dt.bfloat16


def _host_prep(labels, clusters, weights):
    """Per-core pair tables (pure label logic, no embedding data)."""
    labels = np.asarray(labels).astype(np.int64)
    clusters = np.asarray(clusters).astype(np.int64)
    weights = np.asarray(weights).astype(np.float32)

    leq = labels[None, :] == labels[:, None]
    rank = np.cumsum(leq.astype(np.int64), axis=1) - 1
    first = leq & (rank % 2 == 1)
    second = leq & (rank % 2 == 0)
    pbase = ~first   # k-mask for in-cluster (i,j) pairs
    qbase = ~second  # k-mask for out-of-cluster pairs

    # balance anchors across cores by pair count (greedy, largest first)
    npos = leq.sum(1) - 1
    order = np.argsort(-npos, kind="stable")
    core_anchors = [[] for _ in range(NCORES)]
    core_load = [0] * NCORES
    for i in order:
        c = int(np.argmin(core_load))
        core_anchors[c].append(int(i))
        core_load[c] += int(npos[i])
    MA = max(len(a) for a in core_anchors)

    all_pairs = []
    for c in range(NCORES):
        pairs = []
        for il, i in enumerate(core_anchors[c]):
            for j in np.where(leq[i])[0]:
                if j != i:
                    pairs.append((il, i, int(j)))
        all_pairs.append(pairs)
    NP = ((max(len(p) for p in all_pairs) + P - 1) // P) * P

    tables = []
    for c in range(NCORES):
        pairs = all_pairs[c]
        sel = np.zeros((MA, NP), np.float32)
        wp1 = np.ones((NP, 1), np.float32)
        madd = np.full((NP, B), NEG, np.float32)
        s_corr = 0.0
        for p, (il, i, j) in enumerate(pairs):
            w = float(weights[labels[j]])
            sel[il, p] = w  # fold per-pair weight into the selector
            wp1[p, 0] = w
            base = pbase[i] if clusters[i] == clusters[j] else qbase[i]
            mask = base.copy()
            mask[i] = False
            madd[p, :] = np.where(mask, 0.0, np.float32(NEG))
            madd[p, j] = 1.0  # sentinel: recovers w*V_ij; host subtracts w+1
            s_corr += w + 1.0
        tables.append(dict(sel=sel, wp1=wp1, madd=madd,
                           anchors=np.array(core_anchors[c], np.int64),
                           s_corr=s_corr, c_corr=float(len(pairs))))
    import ml_dtypes
    for t in tables:
        t["madd"] = t["madd"].astype(ml_dtypes.bfloat16)
    return tables, NP, MA


def _build_program(NP, MA):
    NCH = NP // P          # pair-row chunks
    NDC = D // P           # contraction chunks (4)

    nc = bacc.Bacc("TRN2", target_bir_lowering=False, debug=False,
                   num_devices=NCORES)

    xt = nc.dram_tensor("xt", [D, B], DT, kind="ExternalInput")
    xmyT2 = nc.dram_tensor("xmyT2", [D, MA], DT, kind="ExternalInput")
    sel = nc.dram_tensor("sel", [MA, NP], DT, kind="ExternalInput")
    madd = nc.dram_tensor("madd", [NP, B], BF, kind="ExternalInput")
    wp1v = nc.dram_tensor("wp1v", [NP, 1], DT, kind="ExternalInput")
    out = nc.dram_tensor("out", [1, 3], DT, kind="ExternalOutput")

    with tile.TileContext(nc) as tc:
        with (
            tc.tile_pool(name="cst", bufs=1) as cst,
            tc.tile_pool(name="xtp", bufs=NDC) as xtp,
            tc.tile_pool(name="sq", bufs=4) as sqp,
            tc.tile_pool(name="big", bufs=6) as bigp,
            tc.tile_pool(name="sm", bufs=6) as sm,
            tc.tile_pool(name="ps", bufs=1, space="PSUM") as ps,
            tc.tile_pool(name="vps", bufs=6, space="PSUM") as vps,
        ):
            # --- load inputs (split across both HWDGE queues) ---
            xt_t = []
            xmy_t = []
            for dc in range(NDC):
                tx = xtp.tile([P, B], DT, tag="xt")
                tm = xtp.tile([P, MA], DT, tag="xmy")
                qx = nc.sync if dc % 2 == 0 else nc.scalar
                qm = nc.scalar if dc % 2 == 0 else nc.sync
                qx.dma_start(tx[:], xt[dc * P:(dc + 1) * P, :])
                qm.dma_start(tm[:], xmyT2[dc * P:(dc + 1) * P, :])
                xt_t.append(tx)
                xmy_t.append(tm)

            sel_t = cst.tile([MA, NP], DT)
            nc.scalar.dma_start(sel_t[:], sel[:, :])

            negones = cst.tile([P, MA], DT)
            nc.vector.memset(negones[:], -1.0)
            ones1 = cst.tile([P, 1], DT)
            nc.vector.memset(ones1[:], 1.0)

            # --- V[i,k] = 2 x_i.x_k - |x_k|^2  (PE), interleaved dot/sq ---
            v_psum = ps.tile([MA, B], DT)
            for dc in range(NDC):
                nc.tensor.matmul(v_psum[:], lhsT=xmy_t[dc][:], rhs=xt_t[dc][:],
                                 start=(dc == 0), stop=False)
                xsq = sqp.tile([P, B], DT, tag="xsq")
                nc.vector.tensor_mul(xsq[:], xt_t[dc][:], xt_t[dc][:])
                nc.tensor.matmul(v_psum[:], lhsT=negones[:], rhs=xsq[:],
                                 start=False, stop=(dc == NDC - 1))

            wp1_t = cst.tile([P, NCH], DT)
            nc.scalar.dma_start(
                wp1_t[:], wp1v[:, :].rearrange("(c p) o -> p (c o)", p=P))

            v_sb = cst.tile([MA, B], DT)
            nc.scalar.copy(v_sb[:], v_psum[:])

            # --- per-pair-chunk pipeline ---
            sacc = cst.tile([P, NCH], DT)
            cacc = cst.tile([P, NCH], DT)   # direct counts (even chunks)
            cacc2 = cst.tile([P, NCH], DT)  # sign sums (odd chunks)
            nc.vector.memset(cacc[:], 0.0)
            nc.vector.memset(cacc2[:], 0.0)
            for c in range(NCH):
                # Vg[p,k] = w_p * V[i_p, k] via w-scaled selector matmul
                vg_ps = vps.tile([P, B], DT, tag="vg")
                nc.tensor.matmul(vg_ps[:], lhsT=sel_t[:, c * P:(c + 1) * P],
                                 rhs=v_sb[:], start=True, stop=True)
                mt = bigp.tile([P, B], BF, tag="mt")
                nc.sync.dma_start(mt[:], madd[c * P:(c + 1) * P, :])

                # e[p] = w*V[i_p, j_p]  via (madd > 0.5) * Vg  (PSUM read)
                junk = bigp.tile([P, B], BF, tag="junk")
                e = sm.tile([P, 1], DT, tag="e")
                nc.vector.scalar_tensor_tensor(
                    junk[:], in0=mt[:], scalar=0.5, in1=vg_ps[:],
                    op0=mybir.AluOpType.is_gt, op1=mybir.AluOpType.mult,
                    accum_out=e[:])
                # t = w*V[i_p,:] + madd
                t = bigp.tile([P, B], DT, tag="t")
                nc.vector.tensor_tensor(t[:], vg_ps[:], mt[:],
                                        op=mybir.AluOpType.add)
                # cvec = w - e ;  z = t + cvec
                cvec = sm.tile([P, 1], DT, tag="cvec")
                nc.vector.tensor_scalar(cvec[:], e[:], -1.0, wp1_t[:, c:c + 1],
                                        op0=mybir.AluOpType.mult,
                                        op1=mybir.AluOpType.add)
                # sum(relu(t + cvec)) into sacc[:, c]
                w_tile = bigp.tile([P, B], BF, tag="w")
                nc.scalar.activation(w_tile[:], t[:],
                                     mybir.ActivationFunctionType.Relu,
                                     bias=cvec[:, 0:1], scale=1.0,
                                     accum_out=sacc[:, c:c + 1])
                cl = bigp.tile([P, B], BF, tag="cl")
                if c % 4 == 0:
                    # count = sum(t > e - wp1) into cacc[:, c]  (DVE)
                    cvec2 = sm.tile([P, 1], DT, tag="cvec2")
                    nc.vector.tensor_scalar(cvec2[:], e[:],
                                            wp1_t[:, c:c + 1], None,
                                            op0=mybir.AluOpType.subtract)
                    nc.vector.tensor_scalar(cl[:], t[:], cvec2[:, 0:1], None,
                                            op0=mybir.AluOpType.is_gt,
                                            op1=mybir.AluOpType.add,
                                            accum_out=cacc[:, c:c + 1])
                else:
                    # sum(sign(t + cvec)) into cacc2[:, c]  (ScalarE)
                    nc.scalar.activation(cl[:], t[:],
                                         mybir.ActivationFunctionType.Sign,
                                         bias=cvec[:, 0:1], scale=1.0,
                                         accum_out=cacc2[:, c:c + 1])

            # --- final reduction ---
            red = cst.tile([P, 3], DT)
            nc.vector.tensor_reduce(red[:, 0:1], sacc[:], mybir.AxisListType.X,
                                    mybir.AluOpType.add)
            nc.vector.tensor_reduce(red[:, 1:2], cacc[:], mybir.AxisListType.X,
                                    mybir.AluOpType.add)
            nc.vector.tensor_reduce(red[:, 2:3], cacc2[:], mybir.AxisListType.X,
                                    mybir.AluOpType.add)
            f_psum = ps.tile([1, 3], DT)
            nc.tensor.matmul(f_psum[:], lhsT=ones1[:], rhs=red[:],
                             start=True, stop=True)
            out_sb = cst.tile([1, 3], DT)
            nc.scalar.copy(out_sb[:], f_psum[:])
            nc.sync.dma_start(out[:, :], out_sb[:])

    nc.compile()
    return nc


def _make_in_maps(embeddings, tables, MA):
    x = np.ascontiguousarray(np.asarray(embeddings, dtype=np.float32))
    xt = np.ascontiguousarray(x.T)  # [D, B]
    in_maps = []
    for c in range(NCORES):
        xmy = np.zeros((MA, x.shape[1]), np.float32)
        a = tables[c]["anchors"]
        xmy[:len(a)] = x[a]
        in_maps.append({
            "xt": xt,
            "xmyT2": np.ascontiguousarray(2.0 * xmy.T),
            "sel": tables[c]["sel"],
            "madd": tables[c]["madd"],
            "wp1v": tables[c]["wp1"],
        })
    return in_maps


def run(embeddings, labels, clusters, weights, trace=False):
    tables, NP, MA = _host_prep(labels, clusters, weights)
    nc = _build_program(NP, MA)
    in_maps = _make_in_maps(embeddings, tables, MA)
    res = run_bass_kernel_spmd(nc, in_maps, core_ids=list(range(NCORES)),
                               trace=trace)
    NCH = NP // P
    n_sign = sum(1 for c in range(NCH) if c % 4 != 0)
    S = 0.0
    C = 0.0
    for c, r in enumerate(res.results):
        S += float(r["out"][0, 0]) - tables[c]["s_corr"]
        c_direct = float(r["out"][0, 1])
        c_sign = (float(r["out"][0, 2]) + B * P * n_sign) / 2.0
        C += c_direct + c_sign - tables[c]["c_corr"]
    loss = np.float32(np.float32(S) / np.float32(C + EPS))
    return np.asarray(loss, dtype=np.float32), res


def kernel(embeddings, labels, clusters, weights):
    loss, _ = run(embeddings, labels, clusters, weights)
    return loss



# revision 2
# speedup vs baseline: 1.1393x; 1.1393x over previous
"""Trainium2 Bass kernel for BatchAllTripletWithClustersLossSemiHard (v6).

Strategy (data-parallel over anchors, 8 cores):
  Pairs (i,j) with equal labels are enumerated per core (anchors greedily
  balanced), SORTED BY j so each 128-pair chunk's j-values fall in a <=64
  column window.  Per pair-row p over k=0..B-1:
      z[p,k] = w_p*(1 + V[i_p,k] - V[i_p,j_p]) + madd[p,k]
  with V[i,k] = 2*x_i.x_k - |x_k|^2.  madd in {0, -4096} (fp8e5)
  dead-masks the semi-hard-rank-excluded k plus k==i and k==j, so no
  host-side corrections are needed.

  Device pipeline per chunk: PSUM accumulates selW.V (float32r matmul)
  + I.madd (fp8 matmul); a tiny DVE STT over the 64-col j-window
  extracts e=w*V_ij-4096; cvec=(w-4096)-e (Pool TT) biases a ScalarE
  relu activation that writes bf16 relu values and row-sums into sacc.
  Counts come from batched DVE is_gt passes over the bf16 relu outputs.
  All inputs are host-prepacked into 4 flat DMAs (one per queue, in
  needed-first order); PE is kept busy with warmup matmuls so the
  p-state ramp reaches full clock before the chunk matmuls.  Raw
  [128,n] partials are DMA'd out; the host does the final reduction.
"""

import numpy as np
import ml_dtypes

import concourse.bass as bass
import concourse.tile as tile
from concourse import bacc, mybir
from concourse.bass_utils import run_bass_kernel_spmd

EPS = 1e-8
NEG = -4096.0
B, D, NCORES = 384, 512, 8
P = 128
KW = 64
F32 = mybir.dt.float32
F32R = mybir.dt.float32r
BF = mybir.dt.bfloat16
F8 = mybir.dt.float8e5


def _host_prep(labels, clusters, weights):
    labels = np.asarray(labels).astype(np.int64)
    clusters = np.asarray(clusters).astype(np.int64)
    weights = np.asarray(weights).astype(np.float32)

    leq = labels[None, :] == labels[:, None]
    rank = np.cumsum(leq.astype(np.int64), axis=1) - 1
    first = leq & (rank % 2 == 1)
    second = leq & (rank % 2 == 0)
    pbase = ~first
    qbase = ~second

    npos = leq.sum(1) - 1
    order = np.argsort(-npos, kind="stable")
    core_anchors = [[] for _ in range(NCORES)]
    core_load = [0] * NCORES
    for i in order:
        c = int(np.argmin(core_load))
        core_anchors[c].append(int(i))
        core_load[c] += int(npos[i])
    MA = max(len(a) for a in core_anchors)

    all_pairs = []
    for c in range(NCORES):
        pairs = []
        for il, i in enumerate(core_anchors[c]):
            for j in np.where(leq[i])[0]:
                if j != i:
                    pairs.append((il, i, int(j)))
        pairs.sort(key=lambda t: t[2])  # j-sorted => narrow windows
        all_pairs.append(pairs)
    NP = ((max(len(p) for p in all_pairs) + P - 1) // P) * P
    NCH = NP // P

    # shared j-window starts (SPMD program must be identical across cores)
    kw0s = []
    for ch in range(NCH):
        lo = min(min((t[2] for t in ap[ch * P:(ch + 1) * P]), default=0)
                 for ap in all_pairs)
        kw0s.append(min(lo, B - KW))
    for c in range(NCORES):
        for ch in range(NCH):
            chunk = all_pairs[c][ch * P:(ch + 1) * P]
            if chunk:
                assert max(t[2] for t in chunk) < kw0s[ch] + KW, \
                    "shared window overflow"

    tables = []
    for c in range(NCORES):
        pairs = all_pairs[c]
        sel = np.zeros((MA, NP), np.float32)
        wc = np.full((P, NCH), NEG, np.float32)   # (w + NEG), [p, ch] packed
        madd = np.full((P, NCH, B), NEG, np.float32)
        jwin = np.zeros((P, NCH, KW), np.float32)
        for ch in range(NCH):
            k0 = kw0s[ch]
            for r, (il, i, j) in enumerate(pairs[ch * P:(ch + 1) * P]):
                w = float(weights[labels[j]])
                sel[il, ch * P + r] = w
                wc[r, ch] = w + NEG
                base = pbase[i] if clusters[i] == clusters[j] else qbase[i]
                mask = base.copy()
                mask[i] = False
                mask[j] = False
                madd[r, ch, :] = np.where(mask, 0.0, np.float32(NEG))
                madd[r, ch, j] = NEG
                jwin[r, ch, j - k0] = 1.0
        tables.append(dict(
            sel=sel, wc=wc,
            madd=madd.reshape(P, NCH * B).astype(ml_dtypes.float8_e5m2),
            jwin=jwin.reshape(P, NCH * KW).astype(ml_dtypes.float8_e5m2),
            anchors=np.array(core_anchors[c], np.int64)))
    return tables, NP, MA, kw0s


def _build_program(NP, MA, kw0s):
    NCH = NP // P
    NDC = D // P

    NWARM = 12

    XA = 2 * B
    XB = 2 * B
    MEGA_A = XA + NDC * MA + MA + NCH       # xtA | xmy | negones | wc
    F8LEN = NCH * B + NCH * KW + P          # madd | jwin | ident

    nc = bacc.Bacc("TRN2", target_bir_lowering=False, debug=False,
                   num_devices=NCORES)

    megaA = nc.dram_tensor("megaA", [P, MEGA_A], F32R, kind="ExternalInput")
    megaB = nc.dram_tensor("megaB", [P, XB], F32R, kind="ExternalInput")
    selA = nc.dram_tensor("selA", [MA, NP], F32R, kind="ExternalInput")
    megaF8 = nc.dram_tensor("megaF8", [P, F8LEN], F8, kind="ExternalInput")
    out_s = nc.dram_tensor("out_s", [P, 2 * NCH], F32, kind="ExternalOutput")

    with tile.TileContext(nc) as tc:
        with (
            tc.tile_pool(name="cst", bufs=1) as cst,
            tc.tile_pool(name="sm", bufs=8) as sm,
            tc.tile_pool(name="wps", bufs=1, space="PSUM") as wps,
            tc.tile_pool(name="vps", bufs=1, space="PSUM") as vps,
            tc.tile_pool(name="gps", bufs=6, space="PSUM") as gps,
        ):
            # ---- scratch for PE warmup (memset early, no input dep) ----
            scratch = cst.tile([P, 256], BF)
            nc.vector.memset(scratch[:], 0.25)

            # ---- 4 flat input DMAs, one per queue, needed-first ----
            mA = cst.tile([P, MEGA_A], F32R)
            nc.sync.dma_start(mA[:], megaA[:, :])
            mB = cst.tile([P, XB], F32R)
            nc.scalar.dma_start(mB[:], megaB[:, :])
            mF8 = cst.tile([P, F8LEN], F8)
            nc.gpsimd.dma_start(mF8[:], megaF8[:, :])
            sel_t = cst.tile([MA, NP], F32R)
            nc.scalar.dma_start(sel_t[:, 0:2 * P], selA[:, 0:2 * P])
            nc.scalar.dma_start(sel_t[:, 2 * P:], selA[:, 2 * P:])

            xt_c = [mA[:, c * B:(c + 1) * B] for c in range(2)] + \
                   [mB[:, c * B:(c + 1) * B] for c in range(2)]
            xmy = mA[:, XA:XA + NDC * MA]
            negones = mA[:, XA + NDC * MA:XA + NDC * MA + MA]
            wcbase = XA + NDC * MA + MA
            wc_t = mA[:, wcbase:wcbase + NCH].bitcast(F32)
            madd_t = mF8[:, 0:NCH * B]
            jwin_t = mF8[:, NCH * B:NCH * B + NCH * KW]
            ident_t = mF8[:, NCH * B + NCH * KW:]

            # ---- PE warmup to ramp the clock during DMA wait ----
            warm = wps.tile([P, 256], F32)
            for i in range(NWARM):
                nc.tensor.matmul(warm[:], lhsT=scratch[:, 0:P],
                                 rhs=scratch[:], start=True, stop=True)

            # ---- V = 2*Xa.X - ones*|x|^2 ;  xsq on Scalar/Pool ----
            v_psum = vps.tile([MA, B], F32)
            for dc in range(NDC):
                nc.tensor.matmul(v_psum[:],
                                 lhsT=xmy[:, dc * MA:(dc + 1) * MA],
                                 rhs=xt_c[dc], start=(dc == 0), stop=False)
                xsq = sm.tile([P, B], F32R, tag="xsq")
                if dc % 2 == 0:
                    nc.scalar.activation(
                        xsq[:], xt_c[dc], mybir.ActivationFunctionType.Square)
                else:
                    nc.vector.tensor_tensor(xsq[:], xt_c[dc], xt_c[dc],
                                            op=mybir.AluOpType.mult)
                nc.tensor.matmul(v_psum[:], lhsT=negones,
                                 rhs=xsq[:], start=False, stop=(dc == NDC - 1))

            for i in range(3):
                nc.tensor.matmul(warm[:], lhsT=scratch[:, 0:P],
                                 rhs=scratch[:], start=True, stop=True)
            v_sb = cst.tile([MA, B], F32R)
            nc.scalar.copy(v_sb[:], v_psum[:])

            # ---- per-chunk pipeline ----
            # column layout: S_c at 2c, count_c at 2c+1 (single out tensor)
            sacc_ch = cst.tile([P, 2 * NCH], F32)
            eS = cst.tile([P, NCH], F32)

            for c in range(NCH):
                vg = gps.tile([P, B], F32, tag="vg")
                nc.tensor.matmul(vg[:], lhsT=sel_t[:, c * P:(c + 1) * P],
                                 rhs=v_sb[:], start=True, stop=False)
                nc.tensor.matmul(vg[:], lhsT=ident_t,
                                 rhs=madd_t[:, c * B:(c + 1) * B],
                                 start=False, stop=True)
                # e = w*V_ij + NEG from the 64-col j-window (DVE)
                ej = sm.tile([P, KW], BF, tag="ej")
                k0 = kw0s[c]
                nc.vector.scalar_tensor_tensor(
                    ej[:], in0=jwin_t[:, c * KW:(c + 1) * KW], scalar=0.5,
                    in1=vg[:, k0:k0 + KW],
                    op0=mybir.AluOpType.is_gt, op1=mybir.AluOpType.mult,
                    accum_out=eS[:, c:c + 1])
                # cvec = (w + NEG) - e and its negation (Pool)
                cvec = sm.tile([P, 1], F32, tag="cv")
                nc.gpsimd.tensor_tensor(cvec[:], wc_t[:, c:c + 1],
                                        eS[:, c:c + 1],
                                        op=mybir.AluOpType.subtract)
                ncv = sm.tile([P, 1], F32, tag="ncv")
                nc.gpsimd.tensor_tensor(ncv[:], eS[:, c:c + 1],
                                        wc_t[:, c:c + 1],
                                        op=mybir.AluOpType.subtract)
                # relu+sum (ScalarE) and count (DVE) straight off PSUM
                zl = sm.tile([P, B], BF, tag="zl")
                nc.scalar.activation(zl[:], vg[:],
                                     mybir.ActivationFunctionType.Relu,
                                     bias=cvec[:, 0:1], scale=1.0,
                                     accum_out=sacc_ch[:, 2 * c:2 * c + 1])
                cl = sm.tile([P, B], BF, tag="cl")
                nc.vector.tensor_scalar(cl[:], vg[:], ncv[:, 0:1], None,
                                        op0=mybir.AluOpType.is_gt,
                                        op1=mybir.AluOpType.add,
                                        accum_out=sacc_ch[:, 2 * c + 1:2 * c + 2])
                if c == 6:
                    nc.sync.dma_start(out_s[:, 0:14], sacc_ch[:, 0:14])

            nc.sync.dma_start(out_s[:, 14:], sacc_ch[:, 14:])

    nc.compile()
    return nc


def _make_in_maps(embeddings, tables, NP, MA):
    x = np.ascontiguousarray(np.asarray(embeddings, dtype=np.float32))
    NCH = NP // P
    NDC = D // P
    xt = x.T  # [D, B]
    xtch = [np.ascontiguousarray(xt[dc * P:(dc + 1) * P, :])
            for dc in range(NDC)]
    in_maps = []
    for c in range(NCORES):
        t = tables[c]
        xmy = np.zeros((MA, D), np.float32)
        a = t["anchors"]
        xmy[:len(a)] = 2.0 * x[a]
        xmyT = xmy.T  # [D, MA]
        xmych = [xmyT[dc * P:(dc + 1) * P, :] for dc in range(NDC)]
        megaA = np.concatenate(
            [xtch[0], xtch[1]] + xmych +
            [np.full((P, MA), -1.0, np.float32), t["wc"]], axis=1)
        megaB = np.concatenate([xtch[2], xtch[3]], axis=1)
        megaF8 = np.concatenate(
            [t["madd"], t["jwin"],
             np.eye(P, dtype=ml_dtypes.float8_e5m2)], axis=1)
        in_maps.append({
            "megaA": np.ascontiguousarray(megaA),
            "megaB": np.ascontiguousarray(megaB),
            "selA": t["sel"],
            "megaF8": np.ascontiguousarray(megaF8),
        })
    return in_maps


def run(embeddings, labels, clusters, weights, trace=False):
    tables, NP, MA, kw0s = _host_prep(labels, clusters, weights)
    nc = _build_program(NP, MA, kw0s)
    in_maps = _make_in_maps(embeddings, tables, NP, MA)
    res = run_bass_kernel_spmd(nc, in_maps, core_ids=list(range(NCORES)),
                               trace=trace)
    S = 0.0
    C = 0.0
    for c, r in enumerate(res.results):
        o = np.asarray(r["out_s"], np.float64)
        S += float(o[:, 0::2].sum())
        C += float(o[:, 1::2].sum())
    loss = np.float32(np.float32(S) / np.float32(C + EPS))
    return np.asarray(loss, dtype=np.float32), res


def kernel(embeddings, labels, clusters, weights):
    loss, _ = run(embeddings, labels, clusters, weights)
    return loss
